# revision 8
# baseline (speedup 1.0000x reference)
"""DenseChebConv (theta, nosum) Trainium2 kernel.

Computes out = sum_k theta_k * T_k(L) @ x @ W_k  with the Chebyshev
recurrence T_k = 2 L T_{k-1} - T_{k-2}, for B=4, N=4096, F=32, K=6, O=128.

Strategy (8 NeuronCores):
  - 2 cores per batch; core owns 2048 rows of its batch Laplacian.
  - L rows are pre-transposed + cast to fp16 on host and stay SBUF-resident,
    so HBM reads L exactly once (the recurrence re-reads it 5x from SBUF).
  - Recurrence matmuls: out[f, i] layout, 4x column-tiled PE (F=32).
  - T is kept fp16 end-to-end (validated: rel err 5.4e-4, same as fp32
    master), which shortens the exchange chain by two casts per step.
  - Per-step halves exchange via pairwise AllGather of the transposed fp16
    weights (DRAM bounce; peer = (b0+b1) - own, exact in fp32).  The CC
    trigger and bounce DMAs ride the sync queue (gpsimd wait->trigger
    latency measured ~3us higher).  The exchange is overlapped with
    own-half matmuls of the next step and the deferred projection of the
    previous order.
  - Projection accumulates over all 6 orders in PSUM via row-tiled PE.
"""

import os
import numpy as np

B, N, F, K, O = 4, 4096, 32, 6, 128
NCORES = 8
R = N // 2            # rows per core
S = R // 4            # strip width (psum free dim)
NJC = N // 128        # j-chunks (contraction)

_CACHE = {}


def _build(ncores=NCORES):
    import concourse.bacc as bacc
    import concourse.mybir as mybir
    import concourse.tile as tile

    dt = mybir.dt
    nc = bacc.Bacc("TRN2", target_bir_lowering=False, debug=False,
                   num_devices=ncores)

    LT_in = nc.dram_tensor("LT", [N, R], dt.float16, kind="ExternalInput")
    xhat_in = nc.dram_tensor("xhat", [128, NJC * F], dt.float16,
                             kind="ExternalInput")
    xs16_in = nc.dram_tensor("xs16", [128, S], dt.float16,
                             kind="ExternalInput")
    wrep_in = nc.dram_tensor("wrep", [128, K * O], dt.float16,
                             kind="ExternalInput")
    id_in = nc.dram_tensor("ident", [128, 32], dt.float16,
                           kind="ExternalInput")
    y_out = nc.dram_tensor("y", [128, R], dt.float32, kind="ExternalOutput")

    RG = [[2 * i, 2 * i + 1] for i in range(ncores // 2)]
    SUB = mybir.AluOpType.subtract
    MUL = mybir.AluOpType.mult

    def tp_off(g, c):
        # transpose-psum free offset for natural chunk m = 4g + c (fp16:
        # 1024 elems = one 2KB PSUM bank).  Concurrent PE row-tiles must
        # hit different PSUM banks: bank = g.
        return 1024 * g + 32 * c

    with tile.TileContext(nc) as tc:
        with tc.tile_pool(name="big", bufs=1) as bigp, \
             tc.tile_pool(name="wts", bufs=2) as wpool, \
             tc.tile_pool(name="mst", bufs=3) as mpool, \
             tc.tile_pool(name="twop", bufs=2) as twop, \
             tc.tile_pool(name="zps", bufs=1, space="PSUM") as zpool, \
             tc.tile_pool(name="pps", bufs=1, space="PSUM") as ppool, \
             tc.tile_pool(name="dram", bufs=1, space="DRAM") as dpool:

            # Warm the collective path: absorbs launch skew between the
            # pair while the L stream runs.  Rides gpsimd so it is not
            # stuck behind the L dma_starts on sync.
            dum_sb = bigp.tile([16, 16], dt.float16, name="dum_sb")
            nc.vector.memset(dum_sb[:], 0.0)
            dummy_in = dpool.tile([16, 16], dt.float16, name="cc_dummy_in")
            dummy_out = dpool.tile([32, 16], dt.float16, name="cc_dummy_out")
            nc.gpsimd.dma_start(out=dummy_in[:], in_=dum_sb[:])
            nc.gpsimd.collective_compute(
                "AllGather", mybir.AluOpType.bypass, replica_groups=RG,
                ins=[dummy_in[:].opt()], outs=[dummy_out[:].opt()])

            # Persistent SBUF
            LT = bigp.tile([128, NJC * R], dt.float16, name="LT_sb")
            WrepS = bigp.tile([128, K * O], dt.float16, name="wrep_sb")
            identS = bigp.tile([128, 32], dt.float16, name="ident_sb")
            outS = bigp.tile([128, R], dt.float32, name="out_sb")

            # First L chunks head the sync queue: the stream is the long
            # pole, every cycle before first byte is pure loss.
            for jc in range(4):
                nc.sync.dma_start(
                    out=LT[:, jc * R:(jc + 1) * R],
                    in_=LT_in[128 * jc:128 * (jc + 1), :])

            # x-derived inputs ride the gpsimd queue, parallel to the L
            # stream issue on sync.
            TwfO = wpool.tile([128, 512], dt.float16, tag="twfo", name="twfo0")
            TwfP = wpool.tile([128, 512], dt.float16, tag="twfp", name="twfp0")
            nc.gpsimd.dma_start(out=TwfO[:], in_=xhat_in[:, 0:512])
            nc.gpsimd.dma_start(out=TwfP[:], in_=xhat_in[:, 512:1024])
            Tm = {}
            Tm[0] = mpool.tile([128, S], dt.float16, tag="tm", name="tm0")
            nc.gpsimd.dma_start(out=Tm[0][:], in_=xs16_in[:])
            nc.gpsimd.dma_start(out=WrepS[:], in_=wrep_in[:])
            nc.gpsimd.dma_start(out=identS[:], in_=id_in[:])

            for jc in range(4, NJC):
                nc.sync.dma_start(
                    out=LT[:, jc * R:(jc + 1) * R],
                    in_=LT_in[128 * jc:128 * (jc + 1), :])

            # Projection accumulator: 4 banks, lives across all 6 orders.
            ProjPs = ppool.tile([128, 4 * S], dt.float32, name="proj_ps")

            def proj(k, rhs16, start, stop):
                for g in range(4):
                    nc.tensor.matmul(
                        out=ProjPs[:, S * g:S * (g + 1)],
                        lhsT=WrepS[32 * g:32 * (g + 1), k * O:(k + 1) * O],
                        rhs=rhs16[32 * g:32 * (g + 1), :],
                        start=start, stop=stop,
                        tile_position=(32 * g, 0))

            proj(0, Tm[0], start=True, stop=False)

            def zmm(Z, w, jc, wof):
                for g in range(4):
                    nc.tensor.matmul(
                        out=Z[32 * g:32 * (g + 1), :],
                        lhsT=w[:, wof:wof + F],
                        rhs=LT[:, jc * R + S * g:jc * R + S * (g + 1)],
                        start=(jc == 0), stop=(jc == NJC - 1),
                        skip_group_check=True,
                        tile_position=(0, 32 * g))

            for k in range(1, K):
                # Z[f, i] accumulation over j-chunks; 4 col-tiles = 4 strips
                Z = zpool.tile([128, S], dt.float32, tag="ztp", name=f"z{k}",
                               padded_shape=[128, 2048])
                # own-half chunks overlap the in-flight exchange k-1
                for jc in range(16):
                    zmm(Z, TwfO, jc, jc * F)
                # projection of the previous order keeps PE warm during
                # the CC wait
                if k >= 2:
                    proj(k - 1, Tm[k - 1], start=False, stop=False)
                # peer-half chunks, gated per quarter of the received
                # weights so the first matmuls start as soon as the first
                # 128 columns of TwfP exist
                for jc in range(16, NJC):
                    zmm(Z, TwfP, jc, (jc - 16) * F)

                # T_k master (fp16, [f, i] strips)
                Tm[k] = mpool.tile([128, S], dt.float16, tag="tm",
                                   name=f"tm{k}")
                if k == 1:
                    nc.vector.tensor_copy(Tm[k][:], Z[:])
                else:
                    # T_k = 2 Z - T_{k-2}
                    nc.vector.scalar_tensor_tensor(
                        out=Tm[k][:], in0=Z[:], scalar=2.0, in1=Tm[k - 2][:],
                        op0=MUL, op1=SUB)

                if k < K - 1:
                    # natural-layout fp16 weights for step k+1 (own half):
                    # PE transposes, one psum bank per row-group
                    TPps = zpool.tile([128, 4096], dt.float16, tag="ztp",
                                      name=f"tp{k}")
                    TwfO = wpool.tile([128, 512], dt.float16, tag="twfo",
                                      name=f"twfo{k}")
                    TwfP = wpool.tile([128, 512], dt.float16, tag="twfp",
                                      name=f"twfp{k}")
                    for g in range(4):
                        for c in range(4):
                            o = tp_off(g, c)
                            nc.tensor.transpose(
                                out=TPps[:, o:o + 32],
                                in_=Tm[k][32 * g:32 * (g + 1),
                                          128 * c:128 * (c + 1)],
                                identity=identS[32 * g:32 * (g + 1), :],
                                tile_position=(32 * g, 0))
                    # own weight half for step k+1 (psum -> sbuf)
                    nc.vector.tensor_copy(
                        TwfO[:].rearrange("p (g q) -> p g q", g=4),
                        TPps[:].rearrange("p (g w) -> p g w", g=4)[:, :, 0:128])

                    # exchange halves (pairwise AllGather via DRAM bounce)
                    ag_in = dpool.tile([128, 512], dt.float16,
                                       name=f"ag_in_{k}")
                    ag_out = dpool.tile([256, 512], dt.float16,
                                        name=f"ag_out_{k}")
                    nc.sync.dma_start(out=ag_in[:], in_=TwfO[:])
                    nc.gpsimd.collective_compute(
                        "AllGather", mybir.AluOpType.bypass,
                        replica_groups=RG,
                        ins=[ag_in[:].opt()], outs=[ag_out[:].opt()])
                    # both gathered blocks -> SBUF; peer = (b0 + b1) - own
                    # (exact in fp32: both addends are fp16 values).
                    # Quartered: each 128-col quarter is DMA'd, summed and
                    # subtracted independently, so the first peer matmuls
                    # of the next step start ~3us after the CC lands.
                    AGsb = twop.tile([128, 1024], dt.float16, tag="agsb",
                                     name=f"agsb{k}")
                    SumSb = twop.tile([128, 512], dt.float32, tag="sumsb",
                                      name=f"sumsb{k}")
                    agv = AGsb[:].rearrange("p (r q) -> p r q", r=2)
                    ogv = ag_out[:].rearrange("(r p) q -> p r q", r=2)
                    for q in range(4):
                        cs = slice(128 * q, 128 * (q + 1))
                        nc.sync.dma_start(out=agv[:, :, cs], in_=ogv[:, :, cs])
                        nc.vector.tensor_add(SumSb[:, cs], AGsb[:, cs],
                                             AGsb[:, 512 + 128 * q:
                                                  512 + 128 * (q + 1)])
                        nc.vector.tensor_sub(TwfP[:, cs], SumSb[:, cs],
                                             TwfO[:, cs])
                else:
                    proj(k, Tm[k], start=False, stop=True)

            # strip-pipelined drain: copy strip, DMA strip
            for g in range(4):
                nc.vector.tensor_copy(outS[:, S * g:S * (g + 1)],
                                      ProjPs[:, S * g:S * (g + 1)])
                nc.sync.dma_start(out=y_out[:, S * g:S * (g + 1)],
                                  in_=outS[:, S * g:S * (g + 1)])

    nc.compile()
    return nc


def _host_prep(x, L, W, theta):
    f16, f32 = np.float16, np.float32
    th_w = theta.astype(f32)[:, None, None] * W.astype(f32)   # [K, F, O]
    wrep = np.empty((128, K * O), f16)
    for k in range(K):
        wrep[:, k * O:(k + 1) * O] = np.tile(th_w[k], (4, 1)).astype(f16)
    ident = np.tile(np.eye(32, dtype=f16), (4, 1))

    in_maps = []
    for c in range(NCORES):
        b, h = c // 2, c % 2
        rows = slice(h * R, (h + 1) * R)
        LTfull = L[b][rows, :].T          # [N, R], j global
        LT16 = np.ascontiguousarray(np.concatenate(
            [LTfull[h * R:(h + 1) * R], LTfull[(1 - h) * R:(2 - h) * R]],
            axis=0)).astype(f16)              # own j-half first
        xb = x[b]
        xloc = np.concatenate([xb[h * R:(h + 1) * R],
                               xb[(1 - h) * R:(2 - h) * R]], axis=0)
        xhat = np.ascontiguousarray(
            xloc.reshape(NJC, 128, F).transpose(1, 0, 2)).reshape(128, NJC * F)
        xs = np.ascontiguousarray(
            xb[rows].reshape(4, S, F).transpose(0, 2, 1)).reshape(128, S)
        in_maps.append({
            "LT": LT16,
            "xhat": xhat.astype(f16),
            "xs16": xs.astype(f16),
            "wrep": wrep,
            "ident": ident,
        })
    return in_maps


def kernel(x, L, W, theta):
    from concourse import bass_utils

    x = np.asarray(x, dtype=np.float32)
    L = np.asarray(L, dtype=np.float32)
    W = np.asarray(W, dtype=np.float32)
    theta = np.asarray(theta, dtype=np.float32)

    if "nc" not in _CACHE:
        _CACHE["nc"] = _build()
    nc = _CACHE["nc"]

    in_maps = _host_prep(x, L, W, theta)
    trace = os.environ.get("BASS_CHEB_TRACE", "0") == "1"
    res = bass_utils.run_bass_kernel_spmd(
        nc, in_maps, core_ids=list(range(NCORES)), trace=trace)
    if trace:
        _CACHE["exec_time_ns"] = res.exec_time_ns
        print(f"HW exec time: {res.exec_time_ns} ns")

    out = np.empty((B, N, O), dtype=np.float32)
    for c in range(NCORES):
        b, h = c // 2, c % 2
        out[b, h * R:(h + 1) * R, :] = res.results[c]["y"].T
    return out


# revision 11
# speedup vs baseline: 1.0278x; 1.0278x over previous
"""DenseChebConv (theta, nosum) Trainium2 kernel.

Computes out = sum_k theta_k * T_k(L) @ x @ W_k  with the Chebyshev
recurrence T_k = 2 L T_{k-1} - T_{k-2}, for B=4, N=4096, F=32, K=6, O=128.

Strategy (8 NeuronCores):
  - 2 cores per batch; core owns 2048 rows of its batch Laplacian.
  - L rows are pre-transposed + cast to fp16 on host and stay SBUF-resident,
    so HBM reads L exactly once (the recurrence re-reads it 5x from SBUF).
  - Recurrence matmuls: out[f, i] layout, 4x column-tiled PE (F=32).
  - T is kept fp16 end-to-end (validated: rel err 5.4e-4, same as fp32
    master), which shortens the exchange chain by two casts per step.
  - Per-step halves exchange via pairwise AllGather of the transposed fp16
    weights (DRAM bounce; peer = (b0+b1) - own, exact in fp32).  The CC
    trigger and bounce DMAs ride the sync queue (gpsimd wait->trigger
    latency measured ~3us higher).  The exchange is overlapped with
    own-half matmuls of the next step and the deferred projection of the
    previous order.
  - Projection accumulates over all 6 orders in PSUM via row-tiled PE.
"""

import os
import numpy as np

B, N, F, K, O = 4, 4096, 32, 6, 128
NCORES = 8
R = N // 2            # rows per core
S = R // 4            # strip width (psum free dim)
NJC = N // 128        # j-chunks (contraction)

_CACHE = {}


def _build(ncores=NCORES):
    import concourse.bacc as bacc
    import concourse.mybir as mybir
    import concourse.tile as tile

    dt = mybir.dt
    nc = bacc.Bacc("TRN2", target_bir_lowering=False, debug=False,
                   num_devices=ncores)

    LT_in = nc.dram_tensor("LT", [N, R], dt.float16, kind="ExternalInput")
    xhat_in = nc.dram_tensor("xhat", [128, NJC * F], dt.float16,
                             kind="ExternalInput")
    xs16_in = nc.dram_tensor("xs16", [128, S], dt.float16,
                             kind="ExternalInput")
    wrep_in = nc.dram_tensor("wrep", [128, K * O], dt.float16,
                             kind="ExternalInput")
    id_in = nc.dram_tensor("ident", [128, 32], dt.float16,
                           kind="ExternalInput")
    y_out = nc.dram_tensor("y", [128, R], dt.float32, kind="ExternalOutput")

    RG = [[2 * i, 2 * i + 1] for i in range(ncores // 2)]
    SUB = mybir.AluOpType.subtract
    MUL = mybir.AluOpType.mult

    def tp_off(g, c):
        # transpose-psum free offset for natural chunk m = 4g + c (fp16:
        # 1024 elems = one 2KB PSUM bank).  Concurrent PE row-tiles must
        # hit different PSUM banks: bank = g.
        return 1024 * g + 32 * c

    with tile.TileContext(nc) as tc:
        with tc.tile_pool(name="big", bufs=1) as bigp, \
             tc.tile_pool(name="wts", bufs=2) as wpool, \
             tc.tile_pool(name="mst", bufs=3) as mpool, \
             tc.tile_pool(name="twop", bufs=2) as twop, \
             tc.tile_pool(name="zps", bufs=1, space="PSUM") as zpool, \
             tc.tile_pool(name="pps", bufs=1, space="PSUM") as ppool, \
             tc.tile_pool(name="dram", bufs=1, space="DRAM") as dpool:

            # Warm the collective path: absorbs launch skew between the
            # pair while the L stream runs.  Rides gpsimd so it is not
            # stuck behind the L dma_starts on sync.
            dum_sb = bigp.tile([16, 16], dt.float16, name="dum_sb")
            nc.vector.memset(dum_sb[:], 0.0)
            dummy_in = dpool.tile([16, 16], dt.float16, name="cc_dummy_in")
            dummy_out = dpool.tile([32, 16], dt.float16, name="cc_dummy_out")
            nc.gpsimd.dma_start(out=dummy_in[:], in_=dum_sb[:])
            nc.gpsimd.collective_compute(
                "AllGather", mybir.AluOpType.bypass, replica_groups=RG,
                ins=[dummy_in[:].opt()], outs=[dummy_out[:].opt()])

            # Persistent SBUF
            LT = bigp.tile([128, NJC * R], dt.float16, name="LT_sb")
            WrepS = bigp.tile([128, K * O], dt.float16, name="wrep_sb")
            identS = bigp.tile([128, 32], dt.float16, name="ident_sb")
            outS = bigp.tile([128, R], dt.float32, name="out_sb")

            # First L chunks head the sync queue: the stream is the long
            # pole, every cycle before first byte is pure loss.
            for jc in range(4):
                nc.sync.dma_start(
                    out=LT[:, jc * R:(jc + 1) * R],
                    in_=LT_in[128 * jc:128 * (jc + 1), :])

            # x-derived inputs ride the gpsimd queue, parallel to the L
            # stream issue on sync.
            TwfO = wpool.tile([128, 512], dt.float16, tag="twfo", name="twfo0")
            TwfP = wpool.tile([128, 512], dt.float16, tag="twfp", name="twfp0")
            nc.gpsimd.dma_start(out=TwfO[:], in_=xhat_in[:, 0:512])
            nc.gpsimd.dma_start(out=TwfP[:], in_=xhat_in[:, 512:1024])
            Tm = {}
            Tm[0] = mpool.tile([128, S], dt.float16, tag="tm", name="tm0")
            nc.gpsimd.dma_start(out=Tm[0][:], in_=xs16_in[:])
            nc.gpsimd.dma_start(out=WrepS[:], in_=wrep_in[:])
            nc.gpsimd.dma_start(out=identS[:], in_=id_in[:])

            for jc in range(4, NJC):
                nc.sync.dma_start(
                    out=LT[:, jc * R:(jc + 1) * R],
                    in_=LT_in[128 * jc:128 * (jc + 1), :])

            # Projection accumulator: 4 banks, lives across all 6 orders.
            ProjPs = ppool.tile([128, 4 * S], dt.float32, name="proj_ps")

            def proj(k, rhs16, start, stop):
                for g in range(4):
                    nc.tensor.matmul(
                        out=ProjPs[:, S * g:S * (g + 1)],
                        lhsT=WrepS[32 * g:32 * (g + 1), k * O:(k + 1) * O],
                        rhs=rhs16[32 * g:32 * (g + 1), :],
                        start=start, stop=stop,
                        tile_position=(32 * g, 0))

            proj(0, Tm[0], start=True, stop=False)

            def zmm(Z, w, jc, wof):
                for g in range(4):
                    nc.tensor.matmul(
                        out=Z[32 * g:32 * (g + 1), :],
                        lhsT=w[:, wof:wof + F],
                        rhs=LT[:, jc * R + S * g:jc * R + S * (g + 1)],
                        start=(jc == 0), stop=(jc == NJC - 1),
                        skip_group_check=True,
                        tile_position=(0, 32 * g))

            for k in range(1, K):
                # Z[f, i] accumulation over j-chunks; 4 col-tiles = 4 strips
                Z = zpool.tile([128, S], dt.float32, tag="ztp", name=f"z{k}",
                               padded_shape=[128, 2048])
                # own-half chunks overlap the in-flight exchange k-1
                for jc in range(16):
                    zmm(Z, TwfO, jc, jc * F)
                # projection of the previous order keeps PE warm during
                # the CC wait
                if k >= 2:
                    proj(k - 1, Tm[k - 1], start=False, stop=False)
                # peer-half chunks, gated per quarter of the received
                # weights so the first matmuls start as soon as the first
                # 128 columns of TwfP exist
                for jc in range(16, NJC):
                    zmm(Z, TwfP, jc, (jc - 16) * F)

                # T_k master (fp16, [f, i] strips); quartered along the
                # free dim so the first transposes can start early
                Tm[k] = mpool.tile([128, S], dt.float16, tag="tm",
                                   name=f"tm{k}")
                for q in range(4):
                    cs = slice(128 * q, 128 * (q + 1))
                    if k == 1:
                        nc.vector.tensor_copy(Tm[k][:, cs], Z[:, cs])
                    else:
                        # T_k = 2 Z - T_{k-2}
                        nc.vector.scalar_tensor_tensor(
                            out=Tm[k][:, cs], in0=Z[:, cs], scalar=2.0,
                            in1=Tm[k - 2][:, cs], op0=MUL, op1=SUB)

                if k < K - 1:
                    # natural-layout fp16 weights for step k+1 (own half):
                    # PE transposes, one psum bank per row-group; c-major
                    # order so round c only needs stt quarter c
                    TPps = zpool.tile([128, 4096], dt.float16, tag="ztp",
                                      name=f"tp{k}")
                    TwfO = wpool.tile([128, 512], dt.float16, tag="twfo",
                                      name=f"twfo{k}")
                    TwfP = wpool.tile([128, 512], dt.float16, tag="twfp",
                                      name=f"twfp{k}")
                    for c in range(4):
                        for g in range(4):
                            o = tp_off(g, c)
                            nc.tensor.transpose(
                                out=TPps[:, o:o + 32],
                                in_=Tm[k][32 * g:32 * (g + 1),
                                          128 * c:128 * (c + 1)],
                                identity=identS[32 * g:32 * (g + 1), :],
                                tile_position=(32 * g, 0))
                    # own weight half for step k+1 (psum -> sbuf)
                    nc.vector.tensor_copy(
                        TwfO[:].rearrange("p (g q) -> p g q", g=4),
                        TPps[:].rearrange("p (g w) -> p g w", g=4)[:, :, 0:128])

                    # exchange halves (pairwise AllGather via DRAM bounce)
                    ag_in = dpool.tile([128, 512], dt.float16,
                                       name=f"ag_in_{k}")
                    ag_out = dpool.tile([256, 512], dt.float16,
                                        name=f"ag_out_{k}")
                    nc.scalar.dma_start(out=ag_in[:], in_=TwfO[:])
                    nc.gpsimd.collective_compute(
                        "AllGather", mybir.AluOpType.bypass,
                        replica_groups=RG,
                        ins=[ag_in[:].opt()], outs=[ag_out[:].opt()])
                    # both gathered blocks -> SBUF; peer = (b0 + b1) - own
                    # (exact in fp32: both addends are fp16 values).
                    # Quartered: each 128-col quarter is DMA'd, summed and
                    # subtracted independently, so the first peer matmuls
                    # of the next step start ~3us after the CC lands.
                    AGsb = twop.tile([128, 1024], dt.float16, tag="agsb",
                                     name=f"agsb{k}")
                    SumSb = twop.tile([128, 512], dt.float32, tag="sumsb",
                                      name=f"sumsb{k}")
                    agv = AGsb[:].rearrange("p (r q) -> p r q", r=2)
                    ogv = ag_out[:].rearrange("(r p) q -> p r q", r=2)
                    for q in range(4):
                        cs = slice(128 * q, 128 * (q + 1))
                        eng = nc.sync if q % 2 == 0 else nc.scalar
                        eng.dma_start(out=agv[:, :, cs], in_=ogv[:, :, cs])
                        nc.vector.tensor_add(SumSb[:, cs], AGsb[:, cs],
                                             AGsb[:, 512 + 128 * q:
                                                  512 + 128 * (q + 1)])
                        nc.vector.tensor_sub(TwfP[:, cs], SumSb[:, cs],
                                             TwfO[:, cs])
                else:
                    proj(k, Tm[k], start=False, stop=True)

            # strip-pipelined drain: copy strip, DMA strip
            for g in range(4):
                nc.vector.tensor_copy(outS[:, S * g:S * (g + 1)],
                                      ProjPs[:, S * g:S * (g + 1)])
                nc.sync.dma_start(out=y_out[:, S * g:S * (g + 1)],
                                  in_=outS[:, S * g:S * (g + 1)])

    nc.compile()
    return nc


def _host_prep(x, L, W, theta):
    f16, f32 = np.float16, np.float32
    th_w = theta.astype(f32)[:, None, None] * W.astype(f32)   # [K, F, O]
    wrep = np.empty((128, K * O), f16)
    for k in range(K):
        wrep[:, k * O:(k + 1) * O] = np.tile(th_w[k], (4, 1)).astype(f16)
    ident = np.tile(np.eye(32, dtype=f16), (4, 1))

    in_maps = []
    for c in range(NCORES):
        b, h = c // 2, c % 2
        rows = slice(h * R, (h + 1) * R)
        LTfull = L[b][rows, :].T          # [N, R], j global
        LT16 = np.ascontiguousarray(np.concatenate(
            [LTfull[h * R:(h + 1) * R], LTfull[(1 - h) * R:(2 - h) * R]],
            axis=0)).astype(f16)              # own j-half first
        xb = x[b]
        xloc = np.concatenate([xb[h * R:(h + 1) * R],
                               xb[(1 - h) * R:(2 - h) * R]], axis=0)
        xhat = np.ascontiguousarray(
            xloc.reshape(NJC, 128, F).transpose(1, 0, 2)).reshape(128, NJC * F)
        xs = np.ascontiguousarray(
            xb[rows].reshape(4, S, F).transpose(0, 2, 1)).reshape(128, S)
        in_maps.append({
            "LT": LT16,
            "xhat": xhat.astype(f16),
            "xs16": xs.astype(f16),
            "wrep": wrep,
            "ident": ident,
        })
    return in_maps


def kernel(x, L, W, theta):
    from concourse import bass_utils

    x = np.asarray(x, dtype=np.float32)
    L = np.asarray(L, dtype=np.float32)
    W = np.asarray(W, dtype=np.float32)
    theta = np.asarray(theta, dtype=np.float32)

    if "nc" not in _CACHE:
        _CACHE["nc"] = _build()
    nc = _CACHE["nc"]

    in_maps = _host_prep(x, L, W, theta)
    trace = os.environ.get("BASS_CHEB_TRACE", "0") == "1"
    res = bass_utils.run_bass_kernel_spmd(
        nc, in_maps, core_ids=list(range(NCORES)), trace=trace)
    if trace:
        _CACHE["exec_time_ns"] = res.exec_time_ns
        print(f"HW exec time: {res.exec_time_ns} ns")

    out = np.empty((B, N, O), dtype=np.float32)
    for c in range(NCORES):
        b, h = c // 2, c % 2
        out[b, h * R:(h + 1) * R, :] = res.results[c]["y"].T
    return out
